# revision 1
# baseline (speedup 1.0000x reference)
"""Trainium2 Bass kernel for nn_DAMDiscreteHopfield.

Reference semantics: sequential sweep over perm; at step j, propose flipping
bit i=perm[j] of the state; accept iff energy -sum(relu(W@state)^2) strictly
decreases.  h = W@state is maintained incrementally.

Key host-side reformulation (each index appears exactly once in perm, so the
state value consumed at step j is the INITIAL state[perm[j]]):
    g_j = -2 * state[perm[j]] * W[:, perm[j]]          (precomputable!)
Device-side per step (S = sum(relu(h)^2), accept iff S_new > S):
    hn  = h + g_j
    S_n = sum(max(hn,0)*hn)      (relu^2; per-partition accum + ones-matmul
                                  broadcast-sum so every partition sees S_n)
    a_j = (S_n > S)              (1.0/0.0, replicated on all 128 partitions)
    h   = h + a_j * g_j          (fused scalar_tensor_tensor)
    S   = max(S, S_n)
Output: flip bits a_j; host applies state[perm[j]] *= (1-2*a_j).

The whole sweep is inherently sequential, so it runs on a single core with
G streamed from HBM in chunked 1MB DMAs (memory regime: 128MB total, fully
prefetchable since g_j never depends on the accept decisions).
"""

import numpy as np

import concourse.bacc as bacc
import concourse.mybir as mybir
from concourse.tile import TileContext
from concourse import bass_utils

FP32 = mybir.dt.float32
ALU = mybir.AluOpType

N_PAT = 8192   # rows of W (pattern count)
N_BITS = 4096  # state length == number of sweep steps
P = 128
FREE = N_PAT // P  # 64


def build_program(n_steps: int, cols_per_chunk: int = 32, g_bufs: int = 8):
    """Emit the Bass program for an n_steps-long sweep."""
    assert n_steps % cols_per_chunk == 0
    n_chunks = n_steps // cols_per_chunk
    cf = cols_per_chunk * FREE

    nc = bacc.Bacc()
    gt = nc.dram_tensor("gt", [n_chunks, P, cf], FP32, kind="ExternalInput")
    h0 = nc.dram_tensor("h0", [P, FREE], FP32, kind="ExternalInput")
    flips_out = nc.dram_tensor("flips", [1, n_steps], FP32, kind="ExternalOutput")

    with TileContext(nc) as tc:
        with (
            tc.tile_pool(name="fixed", bufs=1) as fixed,
            tc.tile_pool(name="gpool", bufs=g_bufs) as gpool,
            tc.tile_pool(name="psum", bufs=2, space="PSUM") as psum,
        ):
            ones = fixed.tile([P, P], FP32, tag="ones")
            nc.vector.memset(ones, 1.0)
            h = fixed.tile([P, FREE], FP32, tag="h")
            nc.sync.dma_start(h, h0[:, :])
            hn = fixed.tile([P, FREE], FP32, tag="hn")
            r2 = fixed.tile([P, FREE], FP32, tag="r2")
            sp = fixed.tile([P, 1], FP32, tag="sp")
            S = fixed.tile([P, 1], FP32, tag="S")
            flips = fixed.tile([P, n_steps], FP32, tag="flips")

            # S = sum(relu(h0)^2), replicated across partitions
            nc.vector.scalar_tensor_tensor(r2, h, 0.0, h, ALU.max, ALU.mult,
                                           accum_out=sp)
            ps0 = psum.tile([P, 1], FP32, tag="ps")
            nc.tensor.matmul(ps0, ones, sp, start=True, stop=True)
            nc.vector.tensor_copy(S, ps0)

            for c in range(n_chunks):
                gtile = gpool.tile([P, cf], FP32, tag="g")
                nc.sync.dma_start(gtile, gt[c, :, :])
                for b in range(cols_per_chunk):
                    j = c * cols_per_chunk + b
                    g = gtile[:, b * FREE:(b + 1) * FREE]
                    aj = flips[:, j:j + 1]
                    nc.vector.tensor_tensor(hn, h, g, ALU.add)
                    nc.vector.scalar_tensor_tensor(r2, hn, 0.0, hn, ALU.max,
                                                   ALU.mult, accum_out=sp)
                    ps = psum.tile([P, 1], FP32, tag="ps")
                    nc.tensor.matmul(ps, ones, sp, start=True, stop=True)
                    nc.vector.tensor_tensor(aj, ps, S, ALU.is_gt)
                    nc.vector.tensor_tensor(S, S, ps, ALU.max)
                    nc.vector.scalar_tensor_tensor(h, g, aj, h, ALU.mult,
                                                   ALU.add)

            nc.sync.dma_start(flips_out[:, :], flips[0:1, :])
    nc.finalize()
    return nc


class _Bacc(bacc.Bacc):
    """Bacc minus the move-matmul-waits-to-ldweights pass.

    That pass pins each step's data wait onto the LDWEIGHTS instruction, so
    the ~311ns fp32 ones-reload lands on the serial dependence chain.  With
    the wait left on the matmul (1 wait — within the ISA slot limit, and
    generate_event_semaphores still splits any overflow), the PE sequencer
    runs LDWEIGHTS early, overlapped with the DVE ops of the same step.
    """

    def move_matmul_waits_to_ldweights(self):
        pass


def build_program_v2(n_steps: int, cols_per_chunk: int = 32, g_bufs: int = 8):
    """v2: track only the flip candidate hn (hn_j = h_j + g_j).

        hn_{j+1} = a_j*g_j + (hn_j + d_j),   d_j = g_{j+1} - g_j  (host const)

    The serial chain per step is r2 -> PE -> is_gt -> hn-stt (4 links); the
    Z = hn + d_j add runs in the PE/compare shadow.  All hn values are exact
    small integers in fp32, so results are bit-identical to v1.
    """
    assert n_steps % cols_per_chunk == 0
    n_chunks = n_steps // cols_per_chunk
    cf = cols_per_chunk * FREE

    nc = _Bacc()
    gt = nc.dram_tensor("gt", [n_chunks, P, cf], FP32, kind="ExternalInput")
    dt_ = nc.dram_tensor("dt", [n_chunks, P, cf], FP32, kind="ExternalInput")
    h0 = nc.dram_tensor("h0", [P, FREE], FP32, kind="ExternalInput")
    hn0 = nc.dram_tensor("hn0", [P, FREE], FP32, kind="ExternalInput")
    flips_out = nc.dram_tensor("flips", [1, n_steps], FP32, kind="ExternalOutput")

    with TileContext(nc) as tc:
        with (
            tc.tile_pool(name="fixed", bufs=1) as fixed,
            tc.tile_pool(name="gpool", bufs=g_bufs) as gpool,
            tc.tile_pool(name="dpool", bufs=g_bufs) as dpool,
            tc.tile_pool(name="psum", bufs=2, space="PSUM") as psum,
        ):
            ones = fixed.tile([P, P], FP32, tag="ones")
            nc.vector.memset(ones, 1.0)
            hh = fixed.tile([P, FREE], FP32, tag="hh")
            nc.sync.dma_start(hh, h0[:, :])
            hn = fixed.tile([P, FREE], FP32, tag="hn")
            nc.sync.dma_start(hn, hn0[:, :])
            r2 = fixed.tile([P, FREE], FP32, tag="r2")
            z = fixed.tile([P, FREE], FP32, tag="z")
            sp = fixed.tile([P, 1], FP32, tag="sp")
            S = fixed.tile([P, 1], FP32, tag="S")
            flips = fixed.tile([P, n_steps], FP32, tag="flips")

            # S = sum(relu(h0)^2), replicated across partitions
            nc.vector.scalar_tensor_tensor(r2, hh, 0.0, hh, ALU.max, ALU.mult,
                                           accum_out=sp)
            ps0 = psum.tile([P, 1], FP32, tag="ps")
            nc.tensor.matmul(ps0, ones, sp, start=True, stop=True)
            nc.vector.tensor_copy(S, ps0)

            for c in range(n_chunks):
                gtile = gpool.tile([P, cf], FP32, tag="g")
                nc.sync.dma_start(gtile, gt[c, :, :])
                dtile = dpool.tile([P, cf], FP32, tag="d")
                nc.sync.dma_start(dtile, dt_[c, :, :])
                for b in range(cols_per_chunk):
                    j = c * cols_per_chunk + b
                    g = gtile[:, b * FREE:(b + 1) * FREE]
                    d = dtile[:, b * FREE:(b + 1) * FREE]
                    aj = flips[:, j:j + 1]
                    nc.vector.scalar_tensor_tensor(r2, hn, 0.0, hn, ALU.max,
                                                   ALU.mult, accum_out=sp)
                    nc.vector.tensor_tensor(z, hn, d, ALU.add)
                    ps = psum.tile([P, 1], FP32, tag="ps")
                    nc.tensor.matmul(ps, ones, sp, start=True, stop=True)
                    nc.vector.tensor_tensor(aj, ps, S, ALU.is_gt)
                    nc.vector.scalar_tensor_tensor(hn, g, aj, z, ALU.mult,
                                                   ALU.add)
                    # S-update issues after the commit: it executes inside the
                    # next r2-stt's unavoidable RAW wait on hn, off the chain.
                    nc.vector.tensor_tensor(S, S, ps, ALU.max)

            nc.sync.dma_start(flips_out[:, :], flips[0:1, :])
    nc.finalize()
    return nc


def build_program_v3(n_steps: int, cols_per_chunk: int = 32, g_bufs: int = 8):
    """v3: like v2, but the cross-partition broadcast-sum uses gpsimd
    partition_all_reduce (405ns, SBUF->SBUF, one op) instead of the PE
    ones-matmul (LDWEIGHTS+MATMUL ~671ns serial, since walrus pins the data
    wait on the fp32 weight reload).  No PSUM involved.
    """
    from concourse import bass_isa
    assert n_steps % cols_per_chunk == 0
    n_chunks = n_steps // cols_per_chunk
    cf = cols_per_chunk * FREE

    nc = bacc.Bacc()
    gt = nc.dram_tensor("gt", [n_chunks, P, cf], FP32, kind="ExternalInput")
    dt_ = nc.dram_tensor("dt", [n_chunks, P, cf], FP32, kind="ExternalInput")
    h0 = nc.dram_tensor("h0", [P, FREE], FP32, kind="ExternalInput")
    hn0 = nc.dram_tensor("hn0", [P, FREE], FP32, kind="ExternalInput")
    flips_out = nc.dram_tensor("flips", [1, n_steps], FP32, kind="ExternalOutput")

    with TileContext(nc) as tc:
        with (
            tc.tile_pool(name="fixed", bufs=1) as fixed,
            tc.tile_pool(name="gpool", bufs=g_bufs) as gpool,
            tc.tile_pool(name="dpool", bufs=g_bufs) as dpool,
        ):
            hh = fixed.tile([P, FREE], FP32, tag="hh")
            nc.sync.dma_start(hh, h0[:, :])
            hn = fixed.tile([P, FREE], FP32, tag="hn")
            nc.sync.dma_start(hn, hn0[:, :])
            r2 = fixed.tile([P, FREE], FP32, tag="r2")
            z = fixed.tile([P, FREE], FP32, tag="z")
            sp = fixed.tile([P, 1], FP32, tag="sp")
            sr = fixed.tile([P, 1], FP32, tag="sr")
            S = fixed.tile([P, 1], FP32, tag="S")
            flips = fixed.tile([P, n_steps], FP32, tag="flips")

            # S = sum(relu(h0)^2), replicated across partitions
            nc.vector.scalar_tensor_tensor(r2, hh, 0.0, hh, ALU.max, ALU.mult,
                                           accum_out=sp)
            nc.gpsimd.partition_all_reduce(S, sp, 128, bass_isa.ReduceOp.add)

            for c in range(n_chunks):
                gtile = gpool.tile([P, cf], FP32, tag="g")
                nc.sync.dma_start(gtile, gt[c, :, :])
                dtile = dpool.tile([P, cf], FP32, tag="d")
                nc.sync.dma_start(dtile, dt_[c, :, :])
                for b in range(cols_per_chunk):
                    j = c * cols_per_chunk + b
                    g = gtile[:, b * FREE:(b + 1) * FREE]
                    d = dtile[:, b * FREE:(b + 1) * FREE]
                    aj = flips[:, j:j + 1]
                    nc.vector.scalar_tensor_tensor(r2, hn, 0.0, hn, ALU.max,
                                                   ALU.mult, accum_out=sp)
                    nc.vector.tensor_tensor(z, hn, d, ALU.add)
                    nc.gpsimd.partition_all_reduce(sr, sp, 128,
                                                   bass_isa.ReduceOp.add)
                    nc.vector.tensor_tensor(aj, sr, S, ALU.is_gt)
                    nc.vector.scalar_tensor_tensor(hn, g, aj, z, ALU.mult,
                                                   ALU.add)
                    nc.vector.tensor_tensor(S, S, sr, ALU.max)

            nc.sync.dma_start(flips_out[:, :], flips[0:1, :])
    nc.finalize()
    return nc


def _chunk_tile(A: np.ndarray, n_chunks: int, cols: int) -> np.ndarray:
    return np.ascontiguousarray(
        A.reshape(n_chunks, cols, P, FREE)
         .transpose(0, 2, 1, 3)
         .reshape(n_chunks, P, cols * FREE))


def host_prep(weights: np.ndarray, state: np.ndarray, perm: np.ndarray,
              n_steps: int, cols_per_chunk: int = 32):
    """Build device inputs: chunk-tiled G^T and exact h0."""
    W = np.ascontiguousarray(weights, dtype=np.float32)
    s = np.asarray(state, dtype=np.float32)
    p = np.asarray(perm, dtype=np.int64)[:n_steps]
    sv = s[p]                                       # initial values in visit order
    GT = W.T[p] * (-2.0 * sv)[:, None]              # [n_steps, N_PAT] fp32
    n_chunks = n_steps // cols_per_chunk
    gt = _chunk_tile(GT, n_chunks, cols_per_chunk)
    h0 = np.ascontiguousarray((W @ s).reshape(P, FREE))  # exact ints in fp32
    return gt, h0, sv, p


def host_prep_v2(weights: np.ndarray, state: np.ndarray, perm: np.ndarray,
                 n_steps: int, cols_per_chunk: int = 32):
    """v2 inputs: G stream, D = diff stream, h0 and hn0 = h0 + g_0."""
    W = np.ascontiguousarray(weights, dtype=np.float32)
    s = np.asarray(state, dtype=np.float32)
    p = np.asarray(perm, dtype=np.int64)[:n_steps]
    sv = s[p]
    GT = W.T[p] * (-2.0 * sv)[:, None]              # [n_steps, N_PAT] fp32
    DT = np.empty_like(GT)
    DT[:-1] = GT[1:] - GT[:-1]                      # d_j = g_{j+1} - g_j (exact)
    DT[-1] = 0.0
    n_chunks = n_steps // cols_per_chunk
    gt = _chunk_tile(GT, n_chunks, cols_per_chunk)
    dt_ = _chunk_tile(DT, n_chunks, cols_per_chunk)
    h0v = (W @ s).astype(np.float32)                # exact ints in fp32
    h0 = np.ascontiguousarray(h0v.reshape(P, FREE))
    hn0 = np.ascontiguousarray((h0v + GT[0]).reshape(P, FREE))
    return gt, dt_, h0, hn0, sv, p


def kernel(weights: np.ndarray, state: np.ndarray, perm: np.ndarray) -> np.ndarray:
    n_steps, cols = N_BITS, 32
    gt, dt_, h0, hn0, sv, p = host_prep_v2(weights, state, perm, n_steps, cols)
    nc = build_program_v2(n_steps, cols)
    res = bass_utils.run_bass_kernel_spmd(
        nc, [{"gt": gt, "dt": dt_, "h0": h0, "hn0": hn0}], core_ids=[0])
    a = np.asarray(res.results[0]["flips"]).reshape(-1)[:n_steps]
    out = np.asarray(state, dtype=np.float32).copy()
    out[p] = sv * (1.0 - 2.0 * a.astype(np.float32))
    return out


# ======================== v6: pair-speculation sweep ========================
# Per step s (2 dummy lead-in steps, then one per visited bit):
#   FOLDSCAN  (DVE custom): prefix-sum of relu((h+gpair)*0.5)^2 over [2,64];
#              col 63 = A = branch-0 sum/4, col 127 = A+B1 (B = both).
#   preduce   (gpsimd): all-reduce cols {63,127} across partitions -> [A,B].
#   SMAXSEL   (DVE custom): S_s = max(S_{s-1}, select(S_{s-1}>S_{s-2}, B-A, A))
#   GATED_ADD (DVE custom): h += G_{s-2} * (S_s > S_{s-1})
# gpair_s = [G_{s-2} | G_{s-3}+G_{s-2}] is the host-precomputed fp16 stream
# (speculation: branch sums for step s are computed from h_{s-2}, before the
# step s-1 decision resolves, so the gpsimd reduce overlaps the scalar chain).
# All arithmetic is exact in fp16/fp32 (ints; relu^2/4 sums < 2^24), so the
# accept trace is bit-identical to the reference.  Accepts are recovered on
# the host as S[s] > S[s-1].

from concourse.dve_spec import (Spec, Src0, Src1, C0, C1, Zero, AluOp, relu,
                                sq, select, maxx, Scan, lower as _dve_lower,
                                _has_src1)
from concourse.dve_uop import DveOpSpec as _DveOpSpec
import concourse.dve_ops as _D
from concourse import bass_isa

FP16 = mybir.dt.float16


def _register_op(name, spec, subdim=False):
    if any(o.name == name for o in _D.OPS):
        return next(o for o in _D.OPS if o.name == name)
    idx = len(_D.OPS)
    row = _D._CUSTOM_DVE_ROW_BASE + idx
    assert row < 0x20
    _D._SUB_OPCODE_FOR_NAME[name] = row
    shas = {}
    for ver in ("v3", "v4"):
        uops = _dve_lower(spec, ver=ver)
        shas[ver] = _DveOpSpec(name=name, opcode=row, uops=uops,
                               rd1_en=_has_src1(spec)).sha(ver)
    op = _D.DveOp(name, spec, subdim=subdim, uops_sha=shas)
    _D.OPS.append(op)
    _D.CUSTOM_DVE_SPECS[name] = spec
    return op


def _np_relu(x):
    return np.maximum(x, 0.0)


FOLDSCAN = _register_op(
    "FOLDSCAN_DAM",
    Spec(
        body=Scan(AluOp.ADD, sq(relu((Src0 + Src1) * C0))),
        reference=lambda in0, in1, c0, c1, c2: np.add.accumulate(
            (_np_relu((in0.astype(np.float32) + in1.astype(np.float32)) * c0) ** 2
             ).reshape(in0.shape[0], -1), axis=1),
    ),
)

SMAXSEL = _register_op(
    "SMAXSEL_DAM",
    Spec(
        body=maxx(C0, select(C0 > C1, Src0 - Src1, Src1)),
        reference=lambda in0, in1, c0, c1, c2: np.maximum(
            c0, np.where(c0 > c1, in0 - in1, in1)),
    ),
)

GATED_ADD = _register_op(
    "GATED_ADD_DAM",
    Spec(
        body=select(C0 > C1, Src0 + Src1, Src0 + Zero),
        reference=lambda in0, in1, c0, c1, c2: np.where(
            c0 > c1, in0 + in1, in0 + 0.0),
    ),
)


def build_program_v6(n_steps: int, cols_per_chunk: int = 32, g_bufs: int = 8):
    """n_steps includes the 2 dummy lead-in steps (and any trailing dummies);
    must be divisible by cols_per_chunk."""
    assert n_steps % cols_per_chunk == 0
    n_chunks = n_steps // cols_per_chunk
    cf = cols_per_chunk * 128  # fp16 elems per partition per chunk

    nc = bacc.Bacc()
    gt = nc.dram_tensor("gt", [n_chunks, P, cf], FP16, kind="ExternalInput")
    h0 = nc.dram_tensor("h0", [P, FREE], FP16, kind="ExternalInput")
    sseed = nc.dram_tensor("sseed", [P, 2], FP32, kind="ExternalInput")
    s_out = nc.dram_tensor("strace", [1, n_steps + 2], FP32,
                           kind="ExternalOutput")

    with TileContext(nc) as tc:
        with (
            tc.tile_pool(name="fixed", bufs=1) as fixed,
            tc.tile_pool(name="gpool", bufs=g_bufs) as gpool,
        ):
            h = fixed.tile([P, FREE], FP16, tag="h")
            nc.sync.dma_start(h, h0[:, :])
            S = fixed.tile([P, n_steps + 2], FP32, tag="S")
            nc.sync.dma_start(S[:, 0:2], sseed[:, :])

            h_rep = h[:, :].unsqueeze(1).broadcast_to([P, 2, FREE])

            gtiles = []
            for c in range(n_chunks):
                gtile = gpool.tile([P, cf], FP16, tag="g")
                nc.sync.dma_start(gtile, gt[c, :, :])
                gtiles.append(gtile)

            def g3(s):
                gtile = gtiles[s // cols_per_chunk]
                b = s % cols_per_chunk
                sl = gtile[:, b * 128:(b + 1) * 128]
                return sl.rearrange("p (s n) -> p s n", s=2)

            junk = fixed.tile([1, 1], FP32, tag="junk", name="junk")
            RING = 8
            scans = [fixed.tile([P, 2, FREE], FP32, tag=f"scan{i}", name=f"scan{i}")
                     for i in range(RING)]
            abts = [fixed.tile([P, 2], FP32, tag=f"ab{i}", name=f"ab{i}") for i in range(RING)]

            def fold(s):
                scan = scans[s % RING]
                nc.vector._custom_dve(FOLDSCAN, out=scan[:, :, :], in0=h_rep,
                                      in1=g3(s), s0=0.5, s1=0.0)
                nc.vector.memset(junk, 0.0)
                nc.vector.memset(junk, 0.0)
                ab = abts[s % RING]
                nc.gpsimd.partition_all_reduce(ab[:, :], scan[:, :, 63:64],
                                               128, bass_isa.ReduceOp.add)
                return ab

            abs_ = {}
            abs_[0] = fold(0)
            abs_[1] = fold(1)
            for s in range(n_steps):
                ab = abs_.pop(s)
                nc.vector._custom_dve(SMAXSEL, out=S[:, s + 2:s + 3],
                                      in0=ab[:, 1:2], in1=ab[:, 0:1],
                                      s0=S[:, s + 1:s + 2], s1=S[:, s:s + 1])
                nc.vector._custom_dve(GATED_ADD, out=h[:, :], in0=h[:, :],
                                      in1=g3(s)[:, 0, :],
                                      s0=S[:, s + 2:s + 3],
                                      s1=S[:, s + 1:s + 2])
                if s + 2 < n_steps:
                    abs_[s + 2] = fold(s + 2)

            nc.sync.dma_start(s_out[:, :], S[0:1, :])
    nc.finalize()
    return nc


def host_prep_v6(weights: np.ndarray, state: np.ndarray, perm: np.ndarray,
                 n_real: int, cols_per_chunk: int = 32):
    """Build gpair stream [S_total, P, 2*FREE] fp16, h0 fp16, sseed."""
    W = np.ascontiguousarray(weights, dtype=np.float32)
    s = np.asarray(state, dtype=np.float32)
    p = np.asarray(perm, dtype=np.int64)[:n_real]
    sv = s[p]
    GT = W.T[p] * (-2.0 * sv)[:, None]                 # [n_real, N_PAT] fp32
    s_total = n_real + 2
    pad = (-s_total) % cols_per_chunk
    s_total += pad
    # G in step space: step s uses G_{s-2}; G_{-2}=G_{-1}=0, trailing zeros
    Gs = np.zeros((s_total, N_PAT), np.float32)
    Gs[2:2 + n_real] = GT
    Gprev = np.zeros_like(Gs)                          # G_{s-3}
    Gprev[1:] = np.roll(Gs, 1, axis=0)[1:]
    Gprev[0] = 0.0
    gpair = np.empty((s_total, 2, N_PAT), np.float16)
    gpair[:, 0, :] = Gs
    gpair[:, 1, :] = Gs + Gprev
    # tile to [s_total, P, 2, FREE] -> chunked [n_chunks, P, cols*2*FREE]
    gpair = gpair.reshape(s_total, 2, P, FREE).transpose(0, 2, 1, 3)
    n_chunks = s_total // cols_per_chunk
    gt = np.ascontiguousarray(
        gpair.reshape(n_chunks, cols_per_chunk, P, 2 * FREE)
             .transpose(0, 2, 1, 3)
             .reshape(n_chunks, P, cols_per_chunk * 2 * FREE))
    h0v = (W @ s).astype(np.float32)
    h0 = np.ascontiguousarray(h0v.reshape(P, FREE)).astype(np.float16)
    s0 = float((np.maximum(h0v, 0.0) ** 2).sum() / 4.0)
    sseed = np.full((P, 2), s0, np.float32)
    return gt, h0, sseed, sv, p, s_total


def kernel_v6(weights: np.ndarray, state: np.ndarray, perm: np.ndarray) -> np.ndarray:
    n_real, cols = N_BITS, 32
    gt, h0, sseed, sv, p, s_total = host_prep_v6(weights, state, perm, n_real, cols)
    nc = build_program_v6(s_total, cols)
    res = bass_utils.run_bass_kernel_spmd(
        nc, [{"gt": gt, "h0": h0, "sseed": sseed}], core_ids=[0])
    st = np.asarray(res.results[0]["strace"]).reshape(-1)
    a = (st[4:4 + n_real] > st[3:3 + n_real]).astype(np.float32)
    out = np.asarray(state, dtype=np.float32).copy()
    out[p] = sv * (1.0 - 2.0 * a)
    return out

